# revision 5
# baseline (speedup 1.0000x reference)
"""FPLPGCN (2x GCNConv feature prop + 10x label prop + fuse) on 8 trn2 cores.

Strategy (graph/data parallel, per sharding hint):
- Nodes are assigned table ranks so that (a) each core owns NPAD=12544 rows
  (44 zero "fake" rows pad 100000 -> 100352), (b) each node's in-edge sources
  are balanced across the 4 table chunks (= core pairs) via a greedy
  assignment, and (c) nodes with similar per-chunk in-edge counts share a
  window band, which makes the padded gather-slot grid tight (~1.24x the
  raw edge count instead of 2.2x).
- GCN math is refactored so aggregation is a pure gather+sum:
      out[n] = dinv[n] * (sum_{e->n} u'[src_e] + u'[n]) + b,
  with u' = dinv * (z @ W) the "table" that is AllGather'ed each round.
- Per conv round: each core computes u' for its nodes (PE matmul), AllGathers
  the table to HBM, then aggregates its in-edges with bulk dma_gather calls
  (node-aligned slot grids, int16 indices, 4 chunk base offsets) and DVE
  strided reduces.  The self term u'[n] stays resident in SBUF.
"""

import sys

sys.path.insert(0, "/opt/trn_rl_repo")

import numpy as np

NC = 8
P = 128
NPAD = 12544          # local rows per core (98 windows of 128)
NWIN = NPAD // P      # 98
TABROWS = NC * NPAD   # 100352
CH = TABROWS // 4     # 25088 chunk stride (= rows of one core pair)
SPAN = 6              # windows per gather-call group
IN_DIM, HID, OUT, DW = 128, 64, 32, 64
NUM_LBL = 10
FSLOT = 64            # gather element = 64 f32 = 256B (labels pad 32->64)


# ----------------------------------------------------------------------------
# host-side index preprocessing (pure index manipulation; no FP math on data)
# ----------------------------------------------------------------------------

def _assign_ranks(src, dst, n_nodes):
    """Choose the node -> table-rank map.

    chunk(table row r) = r // CH = core_of // 2.  Greedily pick each node's
    chunk so that every destination's in-edge sources split ~deg/4 per chunk,
    then order nodes within each chunk so that nodes with similar per-chunk
    count vectors land in the same window band (window = 1024 consecutive
    ranks across all cores), which minimizes the band-max padded slot count.
    """
    deg = np.bincount(dst, minlength=n_nodes).astype(np.int64)
    o = np.argsort(src, kind="stable")
    dst_by_src = dst[o]
    starts = np.searchsorted(src[o], np.arange(n_nodes + 1))
    odeg = starts[1:] - starts[:-1]
    nf = TABROWS - n_nodes

    tgt = deg / 4.0
    capd = np.full(4, CH - nf // 4, np.int64)
    cnt = np.zeros((n_nodes, 4), np.float32)
    chunkof = np.zeros(n_nodes, np.int8)
    for n in np.argsort(-odeg, kind="stable"):
        ds = dst_by_src[starts[n]:starts[n + 1]]
        c = cnt[ds]
        over = np.maximum(c + 1.0 - tgt[ds][:, None], 0.0)
        score = (over * over).sum(axis=0)
        score = np.where(capd > 0, score, np.inf)
        q = int(np.argmin(score))
        chunkof[n] = q
        capd[q] -= 1
        cnt[ds, q] += 1

    cnti = np.zeros((n_nodes, 4), np.int64)
    np.add.at(cnti, (dst, chunkof[src].astype(np.int64)), 1)
    peak = cnti.max(axis=1)
    am = cnti.argmax(axis=1)
    snd = np.sort(cnti, axis=1)[:, 2]
    key = peak * 1000 + am * 200 + snd

    rank = np.empty(n_nodes, np.int64)
    for q in range(4):
        nq = np.flatnonzero(chunkof == q)
        srt = nq[np.argsort(key[nq], kind="stable")]
        pos = np.arange(len(srt)) + nf // 4      # fakes take pos 0..nf/4-1
        rank[srt] = (pos // 2) * 8 + 2 * q + (pos % 2)
    return rank, deg


def _preprocess(edge_index, n_nodes):
    src = np.asarray(edge_index[0], dtype=np.int64)
    dst = np.asarray(edge_index[1], dtype=np.int64)
    E = src.shape[0]

    rank, deg = _assign_ranks(src, dst, n_nodes)
    core_of = rank % NC
    local_of = rank // NC
    trow = core_of * NPAD + local_of                # table row per real node

    # per-edge placement
    ecore = core_of[dst]
    eloc = local_of[dst]
    ewin = eloc // P
    epart = eloc % P
    etr_s = trow[src]
    echunk = etr_s // CH

    # node-chunk slot counts
    nodekey = ecore * NPAD + eloc                   # 0..TABROWS-1
    cnt = np.zeros((TABROWS, 4), np.int32)
    np.add.at(cnt, (nodekey, echunk), 1)

    # common column counts per (window, chunk) across all cores
    K = cnt.reshape(NC, NWIN, P, 4).max(axis=(0, 2))          # [NWIN, 4]

    # k-index of each edge within its (node, chunk) group
    o = np.lexsort((echunk, nodekey))
    nk = nodekey[o]
    ck = echunk[o]
    key = nk * 4 + ck
    first = np.searchsorted(key, key, side="left")
    kidx = np.empty(E, np.int64)
    kidx[o] = np.arange(E) - first

    # span/call layout: spans of SPAN windows; call = (span, chunk)
    nspan = (NWIN + SPAN - 1) // SPAN
    span_of_w = np.arange(NWIN) // SPAN
    # column offset of (w, c) inside call (span(w), c)
    colbase = np.zeros((NWIN, 4), np.int64)
    for s in range(nspan):
        ws = np.arange(s * SPAN, min((s + 1) * SPAN, NWIN))
        for c in range(4):
            acc = 0
            for w in ws:
                colbase[w, c] = acc
                acc += K[w, c]
    call_cols = np.zeros((nspan, 4), np.int64)
    for s in range(nspan):
        ws = slice(s * SPAN, min((s + 1) * SPAN, NWIN))
        call_cols[s] = K[ws].sum(axis=0)
    call_n = call_cols * P                                    # num_idxs per call
    # idx dram layout: calls concatenated (span-major, chunk-minor), each call
    # is n/16 int16 columns wrapped into 16 partitions, replicated x8.
    call_off16 = np.zeros((nspan, 4), np.int64)
    off = 0
    for s in range(nspan):
        for c in range(4):
            call_off16[s, c] = off
            off += call_n[s, c] // 16
    tot16 = off

    # fill idx buffers per core (default 0 -> chunk base row = fake zero row)
    idx_all = np.zeros((NC, 16, tot16), np.int16)
    ci = call_off16[span_of_w[ewin], echunk]                  # per-edge call off
    col = colbase[ewin, echunk] + kidx
    i_in_call = col * P + epart
    rel = etr_s - echunk * CH
    assert rel.min() >= 0 and rel.max() < CH
    r16 = i_in_call % 16
    c16 = ci + i_in_call // 16
    idx_all[ecore, r16, c16] = rel.astype(np.int16)

    meta = dict(core_of=core_of, local_of=local_of,
                trow=trow, deg=deg, K=K, call_cols=call_cols, call_n=call_n,
                call_off16=call_off16, tot16=tot16, nspan=nspan)
    return idx_all, meta


def _shard_nodes(arr, core_of, local_of, width, dtype=np.float32):
    """Scatter full [N, width] node array into per-core [NPAD, width] shards."""
    n = arr.shape[0]
    out = np.zeros((NC, NPAD, width), dtype)
    a2 = np.asarray(arr, dtype).reshape(n, width)
    out[core_of, local_of] = a2
    return out


# ----------------------------------------------------------------------------
# device program
# ----------------------------------------------------------------------------

def _build(meta, nonzero_b):
    import concourse.bacc as bacc
    import concourse.bass as bass
    import concourse.mybir as mybir
    import concourse.tile as tile

    f32 = mybir.dt.float32
    K = meta["K"]
    call_n = meta["call_n"]
    call_off16 = meta["call_off16"]
    tot16 = meta["tot16"]
    nspan = meta["nspan"]

    nc = bacc.Bacc("TRN2", target_bir_lowering=False, debug=False,
                   num_devices=NC, num_swdge_queues=4)

    x_sh = nc.dram_tensor("x_sh", [NPAD, IN_DIM], f32, kind="ExternalInput")
    y_sh = nc.dram_tensor("y_sh", [NPAD, OUT], f32, kind="ExternalInput")
    dw_sh = nc.dram_tensor("dw_sh", [NPAD, DW], f32, kind="ExternalInput")
    mask_sh = nc.dram_tensor("mask_sh", [NPAD, 1], mybir.dt.int8,
                             kind="ExternalInput")
    deg_sh = nc.dram_tensor("deg_sh", [NPAD, 1], mybir.dt.int32,
                            kind="ExternalInput")
    idx_d = nc.dram_tensor("idx_d", [P, tot16], mybir.dt.int16,
                           kind="ExternalInput")
    W0_d = nc.dram_tensor("W0", [IN_DIM, HID], f32, kind="ExternalInput")
    W1_d = nc.dram_tensor("W1", [HID, HID], f32, kind="ExternalInput")
    Wl_d = nc.dram_tensor("Wl", [NUM_LBL * OUT, OUT], f32, kind="ExternalInput")
    Wf_d = nc.dram_tensor("Wf", [HID + OUT + DW, OUT], f32, kind="ExternalInput")
    b_d = nc.dram_tensor("b_all", [4, max(HID, OUT) * NUM_LBL], f32,
                         kind="ExternalInput")  # b0|b1|bl(10x32)|bf rows
    out_sh = nc.dram_tensor("out_sh", [NPAD, OUT], f32, kind="ExternalOutput")

    # internal DRAM
    tabF = [nc.dram_tensor(f"tabF{i}", [TABROWS, FSLOT], f32,
                           addr_space="Shared") for i in range(2)]
    tabL = [nc.dram_tensor(f"tabL{i}", [TABROWS, FSLOT], f32,
                           addr_space="Shared") for i in range(2)]
    bnF = [nc.dram_tensor(f"bnF{i}", [NPAD, FSLOT], f32) for i in range(2)]
    bnL = [nc.dram_tensor(f"bnL{i}", [NPAD, FSLOT], f32) for i in range(2)]

    with tile.TileContext(nc) as tc:
        with tc.tile_pool(name="persist", bufs=1) as pp, \
             tc.tile_pool(name="g", bufs=2) as gp, \
             tc.tile_pool(name="ix", bufs=2) as ixp, \
             tc.tile_pool(name="wk", bufs=3) as wk, \
             tc.tile_pool(name="ps", bufs=2, space="PSUM") as ps:

            # ---- constants / persistent state ----
            W0 = pp.tile([IN_DIM, HID], f32); nc.sync.dma_start(out=W0[:], in_=W0_d[:, :])
            # W1 / Wl replicated along partition offsets so batched-transpose
            # lhsT slices (base partition 64a / 32a) see rhs at the same base.
            W1 = pp.tile([P, HID], f32)
            for a in range(P // HID):
                nc.sync.dma_start(out=W1[a * HID:(a + 1) * HID, :], in_=W1_d[:, :])
            Wl = pp.tile([P, NUM_LBL * OUT], f32)
            for j in range(NUM_LBL):
                for a in range(P // OUT):
                    nc.sync.dma_start(
                        out=Wl[a * OUT:(a + 1) * OUT, j * OUT:(j + 1) * OUT],
                        in_=Wl_d[j * OUT:(j + 1) * OUT, :])
            Wfa = pp.tile([128, OUT], f32); nc.sync.dma_start(out=Wfa[:], in_=Wf_d[0:128, :])
            Wfb = pp.tile([HID + OUT + DW - 128, OUT], f32)
            nc.sync.dma_start(out=Wfb[:], in_=Wf_d[128:, :])
            from concourse.masks import make_identity
            ident = pp.tile([P, P], f32)
            make_identity(nc, ident[:])

            yb = pp.tile([P, NWIN * OUT], f32)
            nc.sync.dma_start(
                out=yb[:].rearrange("p (w f) -> p w f", w=NWIN),
                in_=y_sh[:, :].rearrange("(w p) f -> p w f", p=P))
            maskb = pp.tile([P, NWIN], mybir.dt.int8)
            nc.sync.dma_start(
                out=maskb[:],
                in_=mask_sh[:, 0].rearrange("(w p) -> p w", p=P))
            degb = pp.tile([P, NWIN], mybir.dt.int32)
            nc.sync.dma_start(
                out=degb[:],
                in_=deg_sh[:, 0].rearrange("(w p) -> p w", p=P))

            degf = pp.tile([P, NWIN], f32)
            nc.vector.tensor_copy(out=degf[:], in_=degb[:])
            recipb = pp.tile([P, NWIN], f32)
            nc.vector.tensor_scalar(out=degf[:], in0=degf[:], scalar1=1.0,
                                    scalar2=None, op0=mybir.AluOpType.add)
            nc.vector.reciprocal(out=recipb[:], in_=degf[:])      # 1/(deg+1)
            dinvb = pp.tile([P, NWIN], f32)
            nc.scalar.sqrt(out=dinvb[:], in_=recipb[:])           # 1/sqrt(deg+1)
            # zero fake lanes (window 0, partitions 0..43)
            nc.vector.memset(recipb[0:44, 0:1], 0.0)
            nc.vector.memset(dinvb[0:44, 0:1], 0.0)
            dinvy = pp.tile([P, NWIN * OUT], f32)
            for w in range(NWIN):
                nc.vector.tensor_scalar(
                    out=dinvy[:, w * OUT:(w + 1) * OUT],
                    in0=yb[:, w * OUT:(w + 1) * OUT],
                    scalar1=dinvb[:, w:w + 1], scalar2=None,
                    op0=mybir.AluOpType.mult)

            # bias broadcast tiles (built only when biases are nonzero)
            def bias_tile(row, width):
                bt = pp.tile([P, width], f32, tag=f"bias{row}_{width}", name=f"bias{row}_{width}")
                onecol = pp.tile([1, P], f32, tag="onecol", name="onecol")
                nc.vector.memset(onecol[:], 1.0)
                brow = pp.tile([1, width], f32, tag=f"brow{row}_{width}", name=f"brow{row}_{width}")
                nc.sync.dma_start(out=brow[:], in_=b_d[row:row + 1, 0:width])
                pt = ps.tile([P, width], f32, tag="biasps", name="biasps")
                nc.tensor.matmul(out=pt[:], lhsT=onecol[:], rhs=brow[:],
                                 start=True, stop=True)
                nc.vector.tensor_copy(out=bt[:], in_=pt[:])
                return bt

            bias0 = bias_tile(0, HID) if nonzero_b[0] else None
            bias1 = bias_tile(1, HID) if nonzero_b[1] else None
            biasf = bias_tile(3, OUT) if nonzero_b[3] else None

            vF = pp.tile([P, NWIN * HID], f32)      # current v (feature chain)
            vL = pp.tile([P, NWIN * OUT], f32)      # current v (label chain)
            hfin = pp.tile([P, NWIN * HID], f32)    # final h (node major)
            xlfin = pp.tile([P, NWIN * OUT], f32)   # final xl (node major)
            uselfF = pp.tile([P, NWIN * HID], f32)  # own u' rows (feature)
            uselfL = pp.tile([P, NWIN * OUT], f32)  # own u' rows (label)

            # ---- helpers ----
            def stage_matmul(vtile, F_in, W_ap, F_out, bounce, uself):
                """u' = v @ W per window -> uself (SBUF) + bounce DRAM."""
                per = min(P // F_in, 3)   # lhsT base partition must be 0/32/64
                for w0 in range(0, NWIN, per):
                    nwt = min(per, NWIN - w0)
                    tp = ps.tile([P, P], f32, tag="tps")
                    nc.tensor.transpose(
                        out=tp[0:nwt * F_in, :],
                        in_=vtile[:, w0 * F_in:(w0 + nwt) * F_in],
                        identity=ident[:])
                    vT = wk.tile([P, P], f32, tag="vT")
                    nc.scalar.copy(out=vT[0:nwt * F_in, :], in_=tp[0:nwt * F_in, :])
                    for a in range(nwt):
                        w = w0 + a
                        up = ps.tile([P, F_out], f32, tag="ups")
                        nc.tensor.matmul(out=up[:],
                                         lhsT=vT[a * F_in:(a + 1) * F_in, :],
                                         rhs=W_ap[a * F_in:(a + 1) * F_in, :],
                                         start=True, stop=True)
                        nc.scalar.copy(out=uself[:, w * F_out:(w + 1) * F_out],
                                       in_=up[:])
                    nc.sync.dma_start(
                        out=bounce[w0 * P:(w0 + nwt) * P, 0:F_out]
                            .rearrange("(w p) f -> p w f", p=P),
                        in_=uself[:, w0 * F_out:(w0 + nwt) * F_out]
                            .rearrange("p (w f) -> p w f", w=nwt))

            def stage_agg(tab, F_out, uself, out_cb):
                """Aggregate: out_cb(w, acc_tile) for each window."""
                qctr = [0]
                for s in range(nspan):
                    w_lo = s * SPAN
                    w_hi = min(w_lo + SPAN, NWIN)
                    ncol16 = int(sum(call_n[s, c] // 16 for c in range(4)))
                    if ncol16 == 0:
                        continue
                    ixt = ixp.tile([P, ncol16], mybir.dt.int16, tag="ix")
                    base16 = int(call_off16[s, 0])
                    nc.sync.dma_start(out=ixt[:],
                                      in_=idx_d[:, base16:base16 + ncol16])
                    span_cols = int(sum(K[w_lo:w_hi, c].sum() for c in range(4)))
                    g = gp.tile([P, span_cols * FSLOT], f32, tag="g")
                    coff = 0
                    reg = {}
                    SUBCOLS = 32      # <=4096 idxs per call (multi-packet)
                    for c in range(4):
                        n = int(call_n[s, c])
                        if n == 0:
                            continue
                        o16 = int(call_off16[s, c]) - base16
                        ncols = n // P
                        for c0 in range(0, ncols, SUBCOLS):
                            c1 = min(c0 + SUBCOLS, ncols)
                            nsub = (c1 - c0) * P
                            nc.gpsimd.dma_gather(
                                out_ap=g[:, (coff + c0) * FSLOT:(coff + c1) * FSLOT]
                                    .rearrange("p (s f) -> p s f", f=FSLOT),
                                in_ap=tab[c * CH:(c + 1) * CH, :],
                                idxs_ap=ixt[:, o16 + c0 * 8:o16 + c1 * 8],
                                num_idxs=nsub, num_idxs_reg=nsub,
                                elem_size=FSLOT, queue_num=qctr[0] % 4,
                                single_packet=False)
                            qctr[0] += 1
                        # record region columns per window
                        cc = coff
                        for w in range(w_lo, w_hi):
                            if K[w, c]:
                                reg.setdefault(w, []).append((cc, int(K[w, c])))
                                cc += int(K[w, c])
                        coff += n // P
                    for w in range(w_lo, w_hi):
                        acc = wk.tile([P, F_out], f32, tag="acc")
                        cur = uself[:, w * F_out:(w + 1) * F_out]
                        first = True
                        for (cstart, ncols) in reg.get(w, []):
                            gv = g[:, cstart * FSLOT:(cstart + ncols) * FSLOT] \
                                .rearrange("p (k f) -> p f k", f=FSLOT)
                            if first:
                                nc.vector.reduce_sum(
                                    out=acc[:], in_=gv[:, 0:F_out, :],
                                    axis=mybir.AxisListType.X)
                                first = False
                            else:
                                t = wk.tile([P, F_out], f32, tag="rt")
                                nc.vector.reduce_sum(
                                    out=t[:], in_=gv[:, 0:F_out, :],
                                    axis=mybir.AxisListType.X)
                                nc.vector.tensor_add(out=acc[:], in0=acc[:],
                                                     in1=t[:])
                        if first:
                            nc.vector.tensor_copy(out=acc[:], in_=cur)
                        else:
                            nc.vector.tensor_add(out=acc[:], in0=acc[:],
                                                 in1=cur)
                        out_cb(w, acc)

            # ---- feature conv 1: v_x = dinv*x ; u_f1 = v_x @ W0 ----
            for w0 in range(0, NWIN, 2):
                nwt = min(2, NWIN - w0)
                for a in range(nwt):
                    w = w0 + a
                    xt = wk.tile([P, IN_DIM], f32, tag="xt")
                    nc.sync.dma_start(out=xt[:], in_=x_sh[w * P:(w + 1) * P, :])
                    nc.vector.tensor_scalar(out=xt[:], in0=xt[:],
                                            scalar1=dinvb[:, w:w + 1],
                                            scalar2=None, op0=mybir.AluOpType.mult)
                    # transpose+matmul inline (F_in=128: one window per transpose)
                    tp = ps.tile([P, P], f32, tag="tps")
                    nc.tensor.transpose(out=tp[:], in_=xt[:], identity=ident[:])
                    vT = wk.tile([P, P], f32, tag="vT")
                    nc.scalar.copy(out=vT[:], in_=tp[:])
                    up = ps.tile([P, HID], f32, tag="ups")
                    nc.tensor.matmul(out=up[:], lhsT=vT[:], rhs=W0[:], start=True,
                                     stop=True)
                    nc.scalar.copy(out=uselfF[:, w * HID:(w + 1) * HID], in_=up[:])
                nc.sync.dma_start(
                    out=bnF[0][w0 * P:(w0 + nwt) * P, 0:HID]
                        .rearrange("(w p) f -> p w f", p=P),
                    in_=uselfF[:, w0 * HID:(w0 + nwt) * HID]
                        .rearrange("p (w f) -> p w f", w=nwt))
            nc.gpsimd.collective_compute(
                "AllGather", bass.mybir.AluOpType.bypass,
                replica_groups=[list(range(NC))],
                ins=[bnF[0][:, :].opt()], outs=[tabF[0][0:TABROWS, :].opt()])

            # ---- label conv 1 input: u_l1 = dinvy @ Wl0 ----
            stage_matmul(dinvy, OUT, Wl[:, 0:OUT], OUT, bnL[0], uselfL)
            nc.gpsimd.collective_compute(
                "AllGather", bass.mybir.AluOpType.bypass,
                replica_groups=[list(range(NC))],
                ins=[bnL[0][:, :].opt()], outs=[tabL[0][0:TABROWS, :].opt()])

            # ---- feature conv 1 aggregate -> v_f1 ; u_f2 ; AG ----
            def cb_f1(w, acc):
                nc.vector.tensor_scalar(out=vF[:, w * HID:(w + 1) * HID],
                                        in0=acc[:], scalar1=recipb[:, w:w + 1],
                                        scalar2=None, op0=mybir.AluOpType.mult)
                if bias0 is not None:
                    dv = wk.tile([P, HID], f32, tag="dbv")
                    nc.vector.tensor_scalar(out=dv[:], in0=bias0[:],
                                            scalar1=dinvb[:, w:w + 1],
                                            scalar2=None,
                                            op0=mybir.AluOpType.mult)
                    nc.vector.tensor_add(out=vF[:, w * HID:(w + 1) * HID],
                                         in0=vF[:, w * HID:(w + 1) * HID],
                                         in1=dv[:])
            stage_agg(tabF[0], HID, uselfF, cb_f1)
            stage_matmul(vF, HID, W1[:, :], HID, bnF[1], uselfF)
            nc.gpsimd.collective_compute(
                "AllGather", bass.mybir.AluOpType.bypass,
                replica_groups=[list(range(NC))],
                ins=[bnF[1][:, :].opt()], outs=[tabF[1][0:TABROWS, :].opt()])

            # ---- label convs 1..10 interleaved with feature conv 2 ----
            def make_label_cb(j):
                last = (j == NUM_LBL)

                def cb(w, acc):
                    if last:
                        dst = xlfin[:, w * OUT:(w + 1) * OUT]
                        nc.vector.tensor_scalar(out=dst, in0=acc[:],
                                                scalar1=dinvb[:, w:w + 1],
                                                scalar2=None,
                                                op0=mybir.AluOpType.mult)
                        nc.vector.copy_predicated(
                            out=dst, mask=maskb[:, w:w + 1].to_broadcast([P, OUT]),
                            data=yb[:, w * OUT:(w + 1) * OUT])
                    else:
                        dst = vL[:, w * OUT:(w + 1) * OUT]
                        nc.vector.tensor_scalar(out=dst, in0=acc[:],
                                                scalar1=recipb[:, w:w + 1],
                                                scalar2=None,
                                                op0=mybir.AluOpType.mult)
                        nc.vector.copy_predicated(
                            out=dst, mask=maskb[:, w:w + 1].to_broadcast([P, OUT]),
                            data=dinvy[:, w * OUT:(w + 1) * OUT])
                return cb

            def cb_f2(w, acc):
                dst = hfin[:, w * HID:(w + 1) * HID]
                nc.vector.tensor_scalar(out=dst, in0=acc[:],
                                        scalar1=dinvb[:, w:w + 1], scalar2=None,
                                        op0=mybir.AluOpType.mult)
                if bias1 is not None:
                    nc.vector.tensor_add(out=dst, in0=dst, in1=bias1[:])

            for j in range(1, NUM_LBL + 1):
                stage_agg(tabL[(j - 1) % 2], OUT, uselfL, make_label_cb(j))
                if j == 1:
                    stage_agg(tabF[1], HID, uselfF, cb_f2)  # feature conv 2
                if j < NUM_LBL:
                    stage_matmul(vL, OUT, Wl[:, j * OUT:(j + 1) * OUT], OUT,
                                 bnL[j % 2], uselfL)
                    nc.gpsimd.collective_compute(
                        "AllGather", bass.mybir.AluOpType.bypass,
                        replica_groups=[list(range(NC))],
                        ins=[bnL[j % 2][:, :].opt()],
                        outs=[tabL[j % 2][0:TABROWS, :].opt()])

            # ---- fuse: sigmoid([h | xl | dw] @ Wf + bf) ----
            for w in range(NWIN):
                dwt = wk.tile([P, DW], f32, tag="dwt")
                nc.sync.dma_start(out=dwt[:], in_=dw_sh[w * P:(w + 1) * P, :])
                fTa = wk.tile([P, P], f32, tag="fTa")
                fTb = wk.tile([DW - 32, P], f32, tag="fTb")
                tp = ps.tile([P, P], f32, tag="tps")
                nc.tensor.transpose(out=tp[0:HID, :],
                                    in_=hfin[:, w * HID:(w + 1) * HID],
                                    identity=ident[:])
                nc.scalar.copy(out=fTa[0:HID, :], in_=tp[0:HID, :])
                tp2 = ps.tile([P, P], f32, tag="tps")
                nc.tensor.transpose(out=tp2[0:OUT, :],
                                    in_=xlfin[:, w * OUT:(w + 1) * OUT],
                                    identity=ident[:])
                nc.scalar.copy(out=fTa[HID:HID + OUT, :], in_=tp2[0:OUT, :])
                tp3 = ps.tile([P, P], f32, tag="tps")
                nc.tensor.transpose(out=tp3[0:DW, :], in_=dwt[:],
                                    identity=ident[:])
                nc.scalar.copy(out=fTa[HID + OUT:P, :],
                               in_=tp3[0:P - HID - OUT, :])
                nc.scalar.copy(out=fTb[:, :], in_=tp3[P - HID - OUT:DW, :])
                op = ps.tile([P, OUT], f32, tag="ops")
                nc.tensor.matmul(out=op[:], lhsT=fTa[:], rhs=Wfa[:],
                                 start=True, stop=False)
                nc.tensor.matmul(out=op[:], lhsT=fTb[:], rhs=Wfb[:],
                                 start=False, stop=True)
                ot = wk.tile([P, OUT], f32, tag="ot")
                if biasf is not None:
                    nc.vector.tensor_add(out=op[:], in0=op[:], in1=biasf[:])
                nc.scalar.activation(out=ot[:], in_=op[:],
                                     func=bass.mybir.ActivationFunctionType.Sigmoid)
                nc.sync.dma_start(out=out_sh[w * P:(w + 1) * P, :], in_=ot[:])

    nc.compile()
    return nc


_CACHE = {}


def kernel(x, y, edge_index, deep_walk_emb, label_input_mask,
           W_gcn0, b_gcn0, W_gcn1, b_gcn1, W_label, b_label, W_fuse, b_fuse):
    import concourse.bass_utils as bass_utils

    n_nodes = x.shape[0]
    ei = np.asarray(edge_index, dtype=np.int64)
    idx_all, meta = _preprocess(ei, n_nodes)
    core_of, local_of = meta["core_of"], meta["local_of"]

    nonzero_b = (bool(np.any(np.asarray(b_gcn0))),
                 bool(np.any(np.asarray(b_gcn1))),
                 bool(np.any(np.asarray(b_label))),
                 bool(np.any(np.asarray(b_fuse))))
    if nonzero_b[2]:
        raise NotImplementedError("nonzero label bias not wired")

    key = ("k2", n_nodes, ei.shape[1], nonzero_b)
    if key not in _CACHE:
        _CACHE[key] = _build(meta, nonzero_b)
    nc = _CACHE[key]

    x_s = _shard_nodes(x, core_of, local_of, IN_DIM)
    y_s = _shard_nodes(y, core_of, local_of, OUT)
    dw_s = _shard_nodes(deep_walk_emb, core_of, local_of, DW)
    mk_s = _shard_nodes(np.asarray(label_input_mask, np.int8)[:, None],
                       core_of, local_of, 1, dtype=np.int8)
    dg_s = np.zeros((NC, NPAD, 1), np.int32)
    dg_s[core_of, local_of, 0] = meta["deg"].astype(np.int32)

    bmax = max(HID, OUT) * NUM_LBL
    b_all = np.zeros((4, bmax), np.float32)
    b_all[0, :HID] = np.asarray(b_gcn0, np.float32)
    b_all[1, :HID] = np.asarray(b_gcn1, np.float32)
    b_all[2, :OUT * NUM_LBL] = np.asarray(b_label, np.float32).reshape(-1)
    b_all[3, :OUT] = np.asarray(b_fuse, np.float32)

    Wl_flat = np.asarray(W_label, np.float32).reshape(NUM_LBL * OUT, OUT)
    idx128 = np.tile(idx_all, (1, 8, 1))   # replicate 16-part wrap to 128

    in_maps = []
    for c in range(NC):
        in_maps.append({
            "x_sh": x_s[c], "y_sh": y_s[c], "dw_sh": dw_s[c],
            "mask_sh": mk_s[c], "deg_sh": dg_s[c],
            "idx_d": idx128[c],
            "W0": np.asarray(W_gcn0, np.float32),
            "W1": np.asarray(W_gcn1, np.float32),
            "Wl": Wl_flat,
            "Wf": np.asarray(W_fuse, np.float32),
            "b_all": b_all,
        })
    res = bass_utils.run_bass_kernel_spmd(nc, in_maps, core_ids=list(range(NC)))
    out = np.empty((n_nodes, OUT), np.float32)
    for c in range(NC):
        sel = core_of == np.int64(c)
        out[sel] = res.results[c]["out_sh"][local_of[sel]]
    return out


# revision 9
# speedup vs baseline: 4.3939x; 4.3939x over previous
"""FPLPGCN (2x GCNConv feature prop + 10x label prop + fuse) on 8 trn2 cores.

v3: graph/data parallel with mask-decomposed label propagation.

Key ideas on top of the gather/AllGather structure:
- The label chain xl_{j+1} = where(mask, y, conv_j(xl_j)) splits into a STATIC
  per-node term and a dynamic one: with dinv-prescaled tables,
     S_j[n] = Cm[n] + Gv_j[n] + vtab_j[n]   (unmasked n; masked outputs unused)
     out_j  = dinv * (S_j @ Wl_j),  vtab_{j+1} = (1-m) * dinv * out_j
  where Cm[n] = sum_e m_s*dinv_s*y_s is static (gathered once inside the first
  feature-conv gather, as extra element columns) and Gv_j only needs edges with
  UNMASKED src AND dst (~E/4) -> label-round gathers shrink ~4x.
- Nodes are ordered masked-first, so label tables cover only the unmasked tail
  of each core's rows (compact AllGathers, 2 int16 chunks).
- Chunk assignment is balanced per destination (greedy) and nodes with similar
  per-chunk count vectors share a window band, keeping gather-slot padding low.
- Pool-engine dma_gather descriptor generation (~7ns/idx) is the bottleneck,
  so total gather indices are minimized above all.
"""

import sys

sys.path.insert(0, "/opt/trn_rl_repo")

import numpy as np

NC = 8
P = 128
NPAD = 12544          # local rows per core (98 windows of 128)
NWIN = NPAD // P      # 98
TABROWS = NC * NPAD   # 100352
CH = TABROWS // 4     # feature chunk stride (= rows of one core pair)
SPAN_F = 2            # windows per feature gather-call group
SPAN_L = 8            # windows per label gather-call group
CAP_F = 71            # max gather-grid columns per feature span (SBUF)
CAP_L = 142           # max gather-grid columns per label span (SBUF)
IN_DIM, HID, OUT, DW = 128, 64, 32, 64
NUM_LBL = 10
WF, WL = 2.0, 9.0     # gather-round weights (feature rounds, label rounds)


# ----------------------------------------------------------------------------
# host-side index preprocessing (pure index manipulation; no FP math on data)
# ----------------------------------------------------------------------------

def _assign_ranks(src, dst, mask, n_nodes):
    """node -> table rank; returns (rank, deg, wcut)."""
    deg = np.bincount(dst, minlength=n_nodes).astype(np.int64)
    o = np.argsort(src, kind="stable")
    dst_by_src = dst[o]
    starts = np.searchsorted(src[o], np.arange(n_nodes + 1))
    nf = TABROWS - n_nodes
    m = mask.astype(bool)
    degL = np.bincount(dst[(~m[src]) & (~m[dst])], minlength=n_nodes).astype(np.int64)

    # --- greedy chunk (= core pair) assignment, balanced per destination ---
    tgtF = deg / 4.0
    tgtL = degL / 2.0            # label chunks = core quads (2 of them)
    capd = np.full(4, CH - nf // 4, np.int64)
    cntF = np.zeros((n_nodes, 4), np.float32)
    cntL = np.zeros((n_nodes, 2), np.float32)
    chunkof = np.zeros(n_nodes, np.int8)
    odeg = starts[1:] - starts[:-1]
    for n in np.argsort(-odeg, kind="stable"):
        ds = dst_by_src[starts[n]:starts[n + 1]]
        cF = cntF[ds]
        ovF = np.maximum(cF + 1.0 - tgtF[ds][:, None], 0.0)
        score = WF * (ovF * ovF).sum(axis=0)
        dsL = None
        if not m[n]:
            dsL = ds[~m[ds]]
            if dsL.size:
                cL = cntL[dsL]
                ovL = np.maximum(cL + 1.0 - tgtL[dsL][:, None], 0.0)
                sL = WL * (ovL * ovL).sum(axis=0)
                score = score + sL[[0, 0, 1, 1]]
        score = np.where(capd > 0, score, np.inf)
        q = int(np.argmin(score))
        chunkof[n] = q
        capd[q] -= 1
        cntF[ds, q] += 1
        if dsL is not None and dsL.size:
            cntL[dsL, q // 2] += 1

    cF = np.zeros((n_nodes, 4), np.int64)
    np.add.at(cF, (dst, chunkof[src].astype(np.int64)), 1)
    selL = (~m[src]) & (~m[dst])
    cL = np.zeros((n_nodes, 2), np.int64)
    np.add.at(cL, (dst[selL], (chunkof[src[selL]] // 2).astype(np.int64)), 1)

    # --- band ordering: masked nodes first, then by count-vector pattern ---
    pF = cF.max(axis=1)
    aF = cF.argmax(axis=1)
    pL = cL.max(axis=1)
    aL = cL.argmax(axis=1)
    unm = (~m).astype(np.int64)
    sort_idx = {}
    masked_cnt = np.zeros(4, np.int64)
    for q in range(4):
        nq = np.flatnonzero(chunkof == q)
        o2 = np.lexsort((aF[nq], pF[nq], aL[nq], pL[nq], unm[nq]))
        sort_idx[q] = nq[o2]
        masked_cnt[q] = int(m[nq].sum()) + nf // 4
    wcut = int((masked_cnt // 256).min())
    assert wcut >= 2, f"mask too sparse for compact label tables ({wcut=})"

    # --- per-band slot costs, then balance bands onto windows ---
    NB = NWIN
    KbF = np.zeros((NB, 4), np.int64)
    KbL = np.zeros((NB, 2), np.int64)
    bposs = {}
    for q in range(4):
        srt = sort_idx[q]
        pos = np.arange(len(srt)) + nf // 4
        bposs[q] = pos
        band = pos // 256
        for c in range(4):
            np.maximum.at(KbF[:, c], band, cF[srt, c])
        for c in range(2):
            np.maximum.at(KbL[:, c], band, cL[srt, c])
    KsF = KbF.sum(axis=1)
    KsL = KbL.sum(axis=1)
    nspF = (NWIN + SPAN_F - 1) // SPAN_F
    nspL = (NWIN + SPAN_L - 1) // SPAN_L
    loadF = np.zeros(nspF)
    loadL = np.zeros(nspL)
    perm = np.full(NB, -1, np.int64)
    free0 = set(range(1, wcut))          # windows for fully-masked bands
    free1 = set(range(wcut, NWIN))       # windows for the rest
    perm[0] = 0
    loadF[0] += KsF[0]
    cost = WF * KsF + WL * KsL

    def place_cost(b, x):
        lf = loadF[x // SPAN_F] + KsF[b]
        ll = loadL[x // SPAN_L] + KsL[b]
        pen = 0.0
        if lf > CAP_F:
            pen += 1e6 * (lf - CAP_F)
        if ll > CAP_L:
            pen += 1e6 * (ll - CAP_L)
        return WF * lf + WL * ll + pen

    order1 = [b for b in np.argsort(-cost) if b >= wcut]
    order0 = [b for b in np.argsort(-KsF) if 1 <= b < wcut]
    for b in order1 + order0:
        cls = free0 if b < wcut else free1
        w = min(cls, key=lambda x: place_cost(b, x))
        cls.remove(w)
        perm[b] = w
        loadF[w // SPAN_F] += KsF[b]
        loadL[w // SPAN_L] += KsL[b]

    # swap-repair: drive per-span loads under the SBUF caps
    band_at = np.empty(NWIN, np.int64)
    band_at[perm] = np.arange(NB)

    def span_loads():
        lf = np.zeros(nspF)
        ll = np.zeros(nspL)
        for w in range(NWIN):
            lf[w // SPAN_F] += KsF[band_at[w]]
            ll[w // SPAN_L] += KsL[band_at[w]]
        return lf, ll
    loadF, loadL = span_loads()
    for _ in range(400):
        sbad = int(np.argmax(loadF))
        over = loadF[sbad] - CAP_F
        lbad = int(np.argmax(loadL))
        overl = loadL[lbad] - CAP_L
        if over <= 0 and overl <= 0:
            break
        if over >= overl:
            ws = [w for w in range(sbad * SPAN_F, min((sbad + 1) * SPAN_F, NWIN))
                  if w != 0]
        else:
            ws = [w for w in range(lbad * SPAN_L, min((lbad + 1) * SPAN_L, NWIN))
                  if w != 0]
        best = None
        for w in ws:
            b = band_at[w]
            cls0 = w < wcut
            for w2 in range(1, NWIN):
                if (w2 < wcut) != cls0 or w2 == w:
                    continue
                b2 = band_at[w2]
                dF = KsF[b2] - KsF[b]
                dL = KsL[b2] - KsL[b]
                nf1 = loadF[w // SPAN_F] + dF
                nf2 = loadF[w2 // SPAN_F] - dF
                nl1 = loadL[w // SPAN_L] + dL
                nl2 = loadL[w2 // SPAN_L] - dL
                if w // SPAN_F == w2 // SPAN_F:
                    nf1 = nf2 = loadF[w // SPAN_F]
                if w // SPAN_L == w2 // SPAN_L:
                    nl1 = nl2 = loadL[w // SPAN_L]
                pen = (max(0, nf1 - CAP_F) + max(0, nf2 - CAP_F)) * 1e3 + \
                      (max(0, nl1 - CAP_L) + max(0, nl2 - CAP_L)) * 1e3 + \
                      max(nf1, nf2) + max(nl1, nl2)
                if best is None or pen < best[0]:
                    best = (pen, w, w2)
        if best is None:
            break
        _, w, w2 = best
        b, b2 = band_at[w], band_at[w2]
        band_at[w], band_at[w2] = b2, b
        loadF, loadL = span_loads()
    perm[band_at] = np.arange(NWIN)

    rank = np.empty(n_nodes, np.int64)
    for q in range(4):
        srt, pos = sort_idx[q], bposs[q]
        wb = perm[pos // 256]
        i = (pos // 2) % 128
        rank[srt] = (wb * 128 + i) * 8 + 2 * q + (pos % 2)
    return rank, deg, wcut


def _layout(e_src, e_dst, etr, chmax, nchunk, core_of, local_of, span):
    """Build the padded gather-slot layout for one edge set / table space.

    etr: per-edge source row in the table's row space; chunk = etr // chmax.
    Returns dict with K, call tables, idx buffers per core.
    """
    E = e_src.shape[0]
    ecore = core_of[e_dst]
    eloc = local_of[e_dst]
    ewin = eloc // P
    epart = eloc % P
    echunk = etr // chmax

    nodekey = ecore * NPAD + eloc
    cnt = np.zeros((TABROWS, nchunk), np.int32)
    np.add.at(cnt, (nodekey, echunk), 1)
    K = cnt.reshape(NC, NWIN, P, nchunk).max(axis=(0, 2))     # [NWIN, nchunk]

    o = np.lexsort((echunk, nodekey))
    nk = nodekey[o]
    ck = echunk[o]
    key = nk * nchunk + ck
    first = np.searchsorted(key, key, side="left")
    kidx = np.empty(E, np.int64)
    kidx[o] = np.arange(E) - first

    nspan = (NWIN + span - 1) // span
    span_of_w = np.arange(NWIN) // span
    colbase = np.zeros((NWIN, nchunk), np.int64)
    for s in range(nspan):
        ws = np.arange(s * span, min((s + 1) * span, NWIN))
        for c in range(nchunk):
            acc = 0
            for w in ws:
                colbase[w, c] = acc
                acc += K[w, c]
    call_cols = np.zeros((nspan, nchunk), np.int64)
    for s in range(nspan):
        ws = slice(s * span, min((s + 1) * span, NWIN))
        call_cols[s] = K[ws].sum(axis=0)
    call_n = call_cols * P
    call_off16 = np.zeros((nspan, nchunk), np.int64)
    off = 0
    for s in range(nspan):
        for c in range(nchunk):
            call_off16[s, c] = off
            off += call_n[s, c] // 16
    tot16 = max(off, 16)

    idx_all = np.zeros((NC, 16, tot16), np.int16)
    ci = call_off16[span_of_w[ewin], echunk]
    col = colbase[ewin, echunk] + kidx
    i_in_call = col * P + epart
    rel = etr - echunk * chmax
    assert rel.min() >= 0 and rel.max() < min(chmax, 32768)
    r16 = i_in_call % 16
    c16 = ci + i_in_call // 16
    idx_all[ecore, r16, c16] = rel.astype(np.int16)

    return dict(K=K, call_cols=call_cols, call_n=call_n,
                call_off16=call_off16, tot16=tot16, nspan=nspan,
                span=span, nchunk=nchunk, idx=idx_all)


def _preprocess(edge_index, mask, n_nodes):
    src = np.asarray(edge_index[0], dtype=np.int64)
    dst = np.asarray(edge_index[1], dtype=np.int64)
    m = np.asarray(mask, bool)

    rank, deg, wcut = _assign_ranks(src, dst, m, n_nodes)
    core_of = rank % NC
    local_of = rank // NC
    trow = core_of * NPAD + local_of

    layF = _layout(src, dst, trow[src], CH, 4, core_of, local_of, SPAN_F)

    base_l = (wcut - 1) * P
    npad_l = NPAD - base_l
    ch_l = 4 * npad_l
    selL = (~m[src]) & (~m[dst])
    sL, dL = src[selL], dst[selL]
    trowL = core_of[sL] * npad_l + (local_of[sL] - base_l)
    assert (local_of[sL] >= wcut * P).all()
    layL = _layout(sL, dL, trowL, ch_l, 2, core_of, local_of, SPAN_L)

    meta = dict(core_of=core_of, local_of=local_of, deg=deg, wcut=wcut,
                base_l=base_l, npad_l=npad_l, ch_l=ch_l, layF=layF, layL=layL)
    return meta


def _shard_nodes(arr, core_of, local_of, width, dtype=np.float32):
    n = arr.shape[0]
    out = np.zeros((NC, NPAD, width), dtype)
    a2 = np.asarray(arr, dtype).reshape(n, width)
    out[core_of, local_of] = a2
    return out


# ----------------------------------------------------------------------------
# device program
# ----------------------------------------------------------------------------

def _build(meta, nonzero_b):
    import concourse.bacc as bacc
    import concourse.bass as bass
    import concourse.mybir as mybir
    import concourse.tile as tile

    f32 = mybir.dt.float32
    layF, layL = meta["layF"], meta["layL"]
    wcut = meta["wcut"]
    base_l = meta["base_l"]
    npad_l = meta["npad_l"]
    ch_l = meta["ch_l"]
    NL = NWIN - wcut
    tabrows_l = NC * npad_l

    nc = bacc.Bacc("TRN2", target_bir_lowering=False, debug=False,
                   num_devices=NC, num_swdge_queues=4)

    x_sh = nc.dram_tensor("x_sh", [NPAD, IN_DIM], f32, kind="ExternalInput")
    y_sh = nc.dram_tensor("y_sh", [NPAD, OUT], f32, kind="ExternalInput")
    dw_sh = nc.dram_tensor("dw_sh", [NPAD, DW], f32, kind="ExternalInput")
    mask_sh = nc.dram_tensor("mask_sh", [NPAD, 1], mybir.dt.int8,
                             kind="ExternalInput")
    deg_sh = nc.dram_tensor("deg_sh", [NPAD, 1], mybir.dt.int32,
                            kind="ExternalInput")
    idxF_d = nc.dram_tensor("idxF_d", [P, layF["tot16"]], mybir.dt.int16,
                            kind="ExternalInput")
    idxL_d = nc.dram_tensor("idxL_d", [P, layL["tot16"]], mybir.dt.int16,
                            kind="ExternalInput")
    W0_d = nc.dram_tensor("W0", [IN_DIM, HID], f32, kind="ExternalInput")
    W1_d = nc.dram_tensor("W1", [HID, HID], f32, kind="ExternalInput")
    Wl_d = nc.dram_tensor("Wl", [NUM_LBL * OUT, OUT], f32, kind="ExternalInput")
    Wf_d = nc.dram_tensor("Wf", [HID + OUT + DW, OUT], f32, kind="ExternalInput")
    b_d = nc.dram_tensor("b_all", [4, max(HID, OUT) * NUM_LBL], f32,
                         kind="ExternalInput")
    out_sh = nc.dram_tensor("out_sh", [NPAD, OUT], f32, kind="ExternalOutput")

    # internal DRAM
    tabR1 = nc.dram_tensor("tabR1", [TABROWS, 128], f32, addr_space="Shared")
    tabF2 = nc.dram_tensor("tabF2", [TABROWS, 64], f32, addr_space="Shared")
    tabL = [nc.dram_tensor(f"tabL{i}", [tabrows_l, 64], f32,
                           addr_space="Shared") for i in range(2)]
    bnR1 = nc.dram_tensor("bnR1", [NPAD, 128], f32)
    bnF2 = nc.dram_tensor("bnF2", [NPAD, 64], f32)
    bnL = [nc.dram_tensor(f"bnL{i}", [NPAD, 64], f32) for i in range(2)]

    # max gather-grid width (f32 elems per partition), shared tile tag
    maxg = max(int(layF["call_cols"].sum(axis=1).max()) * 128,
               int(layL["call_cols"].sum(axis=1).max()) * 64)

    with tile.TileContext(nc) as tc:
        with tc.tile_pool(name="persist", bufs=1) as pp, \
             tc.tile_pool(name="g", bufs=2) as gp, \
             tc.tile_pool(name="ix", bufs=2) as ixp, \
             tc.tile_pool(name="wk", bufs=3) as wk, \
             tc.tile_pool(name="ps", bufs=2, space="PSUM") as ps:

            # ---- constants / persistent state ----
            W0 = pp.tile([IN_DIM, HID], f32); nc.sync.dma_start(out=W0[:], in_=W0_d[:, :])
            W1 = pp.tile([P, HID], f32)
            for a in range(P // HID):
                nc.sync.dma_start(out=W1[a * HID:(a + 1) * HID, :], in_=W1_d[:, :])
            Wl = pp.tile([P, NUM_LBL * OUT], f32)
            for j in range(NUM_LBL):
                for a in range(P // OUT):
                    nc.sync.dma_start(
                        out=Wl[a * OUT:(a + 1) * OUT, j * OUT:(j + 1) * OUT],
                        in_=Wl_d[j * OUT:(j + 1) * OUT, :])
            Wfa = pp.tile([128, OUT], f32); nc.sync.dma_start(out=Wfa[:], in_=Wf_d[0:128, :])
            Wfb = pp.tile([HID + OUT + DW - 128, OUT], f32)
            nc.sync.dma_start(out=Wfb[:], in_=Wf_d[128:, :])
            from concourse.masks import make_identity
            ident = pp.tile([P, P], f32)
            make_identity(nc, ident[:])

            yb = pp.tile([P, NWIN * OUT], f32)
            nc.sync.dma_start(
                out=yb[:].rearrange("p (w f) -> p w f", w=NWIN),
                in_=y_sh[:, :].rearrange("(w p) f -> p w f", p=P))
            maskb = pp.tile([P, NWIN], mybir.dt.int8)
            nc.sync.dma_start(
                out=maskb[:],
                in_=mask_sh[:, 0].rearrange("(w p) -> p w", p=P))
            degb = pp.tile([P, NWIN], mybir.dt.int32)
            nc.sync.dma_start(
                out=degb[:],
                in_=deg_sh[:, 0].rearrange("(w p) -> p w", p=P))

            degf = pp.tile([P, NWIN], f32)
            nc.vector.tensor_copy(out=degf[:], in_=degb[:])
            recipb = pp.tile([P, NWIN], f32)
            nc.vector.tensor_scalar(out=degf[:], in0=degf[:], scalar1=1.0,
                                    scalar2=None, op0=mybir.AluOpType.add)
            nc.vector.reciprocal(out=recipb[:], in_=degf[:])      # 1/(deg+1)
            dinvb = pp.tile([P, NWIN], f32)
            nc.scalar.sqrt(out=dinvb[:], in_=recipb[:])           # 1/sqrt(deg+1)
            nc.vector.memset(recipb[0:44, 0:1], 0.0)
            nc.vector.memset(dinvb[0:44, 0:1], 0.0)
            # umr = (1-m) * recip ; zero where masked
            umr = pp.tile([P, NWIN], f32)
            nc.vector.tensor_copy(out=umr[:], in_=maskb[:])
            nc.vector.tensor_scalar(out=umr[:], in0=umr[:], scalar1=-1.0,
                                    scalar2=1.0, op0=mybir.AluOpType.mult,
                                    op1=mybir.AluOpType.add)
            nc.vector.tensor_mul(out=umr[:], in0=umr[:], in1=recipb[:])
            dinvy = pp.tile([P, NWIN * OUT], f32)
            for w in range(NWIN):
                nc.vector.tensor_scalar(
                    out=dinvy[:, w * OUT:(w + 1) * OUT],
                    in0=yb[:, w * OUT:(w + 1) * OUT],
                    scalar1=dinvb[:, w:w + 1], scalar2=None,
                    op0=mybir.AluOpType.mult)

            # bnR1 static cols: 64:96 = dy (=dinvy), 96:128 = mdy
            nc.sync.dma_start(
                out=bnR1[:, 64:96].rearrange("(w p) f -> p w f", p=P),
                in_=dinvy[:].rearrange("p (w f) -> p w f", w=NWIN))
            GRP = 25
            mdyt = pp.tile([P, GRP * OUT], f32, tag="mdyt", name="mdyt")
            for w0 in range(0, NWIN, GRP):
                nwt = min(GRP, NWIN - w0)
                nc.vector.memset(mdyt[:, 0:nwt * OUT], 0.0)
                for a in range(nwt):
                    w = w0 + a
                    nc.vector.copy_predicated(
                        out=mdyt[:, a * OUT:(a + 1) * OUT],
                        mask=maskb[:, w:w + 1].to_broadcast([P, OUT]),
                        data=dinvy[:, w * OUT:(w + 1) * OUT])
                nc.sync.dma_start(
                    out=bnR1[w0 * P:(w0 + nwt) * P, 96:128]
                        .rearrange("(w p) f -> p w f", p=P),
                    in_=mdyt[:, 0:nwt * OUT].rearrange("p (w f) -> p w f", w=nwt))

            # zero the label-table pad window (window wcut-1) in both bounces
            zt = pp.tile([P, 64], f32, tag="zt", name="zt")
            nc.vector.memset(zt[:], 0.0)
            for i in range(2):
                nc.sync.dma_start(out=bnL[i][base_l:base_l + P, :], in_=zt[:])

            def bias_tile(row, width):
                bt = pp.tile([P, width], f32, tag=f"bias{row}_{width}", name=f"bias{row}_{width}")
                onecol = pp.tile([1, P], f32, tag="onecol", name="onecol")
                nc.vector.memset(onecol[:], 1.0)
                brow = pp.tile([1, width], f32, tag=f"brow{row}_{width}", name=f"brow{row}_{width}")
                nc.sync.dma_start(out=brow[:], in_=b_d[row:row + 1, 0:width])
                pt = ps.tile([P, width], f32, tag="biasps", name="biasps")
                nc.tensor.matmul(out=pt[:], lhsT=onecol[:], rhs=brow[:],
                                 start=True, stop=True)
                nc.vector.tensor_copy(out=bt[:], in_=pt[:])
                return bt

            bias0 = bias_tile(0, HID) if nonzero_b[0] else None
            bias1 = bias_tile(1, HID) if nonzero_b[1] else None
            biasf = bias_tile(3, OUT) if nonzero_b[3] else None

            vFh = pp.tile([P, NWIN * HID], f32)     # vF then (reused) hfin
            uselfF = pp.tile([P, NWIN * HID], f32)  # own u' rows (feature)
            xlfin = pp.tile([P, NWIN * OUT], f32)
            G = pp.tile([P, NL * OUT], f32)         # C_all then Gv_j
            Cstat = pp.tile([P, NL * OUT], f32)     # Cm
            vtab = [pp.tile([P, NL * OUT], f32, tag=f"vtab{i}", name=f"vtab{i}")
                    for i in range(2)]

            # xlfin = y for the fully-masked windows
            for w in range(wcut):
                nc.vector.tensor_copy(out=xlfin[:, w * OUT:(w + 1) * OUT],
                                      in_=yb[:, w * OUT:(w + 1) * OUT])

            # ---- helpers ----
            def stage_matmul(vsrc, F_in, W_ap, F_out, windows, post_cb, tgt,
                             tgt_off, bounce, bcol):
                """per window in `windows`: u = vsrc(w) @ W -> post_cb -> tgt
                tile; group-DMA tgt -> bounce rows (cols bcol:bcol+F_out)."""
                per = min(P // F_in, 3)
                wl0 = 0
                wlist = list(windows)
                while wl0 < len(wlist):
                    nwt = min(per, len(wlist) - wl0)
                    w0 = wlist[wl0]
                    tp = ps.tile([P, P], f32, tag="tps")
                    nc.tensor.transpose(
                        out=tp[0:nwt * F_in, :],
                        in_=vsrc(w0, nwt),
                        identity=ident[:])
                    vT = wk.tile([P, P], f32, tag="vT")
                    nc.scalar.copy(out=vT[0:nwt * F_in, :], in_=tp[0:nwt * F_in, :])
                    for a in range(nwt):
                        w = w0 + a
                        up = ps.tile([P, F_out], f32, tag="ups")
                        nc.tensor.matmul(out=up[:],
                                         lhsT=vT[a * F_in:(a + 1) * F_in, :],
                                         rhs=W_ap[a * F_in:(a + 1) * F_in, :],
                                         start=True, stop=True)
                        post_cb(w, up)
                    if bounce is not None:
                        tl0 = w0 - tgt_off
                        nc.sync.dma_start(
                            out=bounce[w0 * P:(w0 + nwt) * P, bcol:bcol + F_out]
                                .rearrange("(w p) f -> p w f", p=P),
                            in_=tgt[:, tl0 * F_out:(tl0 + nwt) * F_out]
                                .rearrange("p (w f) -> p w f", w=nwt))
                    wl0 += nwt

            def stage_agg(tab, lay, fslot, fields, idx_d):
                """fields: list of (lo, hi, self_tile, F_self, wmin, cb)."""
                K = lay["K"]
                call_n = lay["call_n"]
                call_off16 = lay["call_off16"]
                nchunk = lay["nchunk"]
                span = lay["span"]
                chmax = ch_l if lay is layL else CH
                qctr = [0]
                for s in range(lay["nspan"]):
                    w_lo = s * span
                    w_hi = min(w_lo + span, NWIN)
                    ncol16 = int(sum(call_n[s, c] // 16 for c in range(nchunk)))
                    if ncol16 == 0:
                        continue
                    ixt = ixp.tile([P, ncol16], mybir.dt.int16, tag="ix")
                    base16 = int(call_off16[s, 0])
                    nc.sync.dma_start(out=ixt[:],
                                      in_=idx_d[:, base16:base16 + ncol16])
                    span_cols = int(sum(K[w_lo:w_hi, c].sum()
                                        for c in range(nchunk)))
                    g = gp.tile([P, maxg], f32, tag="g")
                    coff = 0
                    reg = {}
                    SUBN = 4096 // (fslot // 64)   # <=4096 desc equivalents
                    for c in range(nchunk):
                        n = int(call_n[s, c])
                        if n == 0:
                            continue
                        o16 = int(call_off16[s, c]) - base16
                        ncols = n // P
                        percall = max(SUBN // P, 1)
                        for c0 in range(0, ncols, percall):
                            c1 = min(c0 + percall, ncols)
                            nsub = (c1 - c0) * P
                            nc.gpsimd.dma_gather(
                                out_ap=g[:, (coff + c0) * fslot:(coff + c1) * fslot]
                                    .rearrange("p (s f) -> p s f", f=fslot),
                                in_ap=tab[c * chmax:(c + 1) * chmax, :],
                                idxs_ap=ixt[:, o16 + c0 * 8:o16 + c1 * 8],
                                num_idxs=nsub, num_idxs_reg=nsub,
                                elem_size=fslot, queue_num=qctr[0] % 4,
                                single_packet=False)
                            qctr[0] += 1
                        cc = coff
                        for w in range(w_lo, w_hi):
                            if K[w, c]:
                                reg.setdefault(w, []).append((cc, int(K[w, c])))
                                cc += int(K[w, c])
                        coff += ncols
                    for w in range(w_lo, w_hi):
                        regs = reg.get(w, [])
                        for (lo, hi, selft, F_self, wmin, cb) in fields:
                            if w < wmin:
                                continue
                            F_out = hi - lo
                            acc = wk.tile([P, F_out], f32, tag="acc")
                            first = True
                            for (cstart, ncols) in regs:
                                gv = g[:, cstart * fslot:(cstart + ncols) * fslot] \
                                    .rearrange("p (k f) -> p f k", f=fslot)
                                if first:
                                    nc.vector.reduce_sum(
                                        out=acc[:], in_=gv[:, lo:hi, :],
                                        axis=mybir.AxisListType.X)
                                    first = False
                                else:
                                    t = wk.tile([P, F_out], f32, tag="rt")
                                    nc.vector.reduce_sum(
                                        out=t[:], in_=gv[:, lo:hi, :],
                                        axis=mybir.AxisListType.X)
                                    nc.vector.tensor_add(out=acc[:], in0=acc[:],
                                                         in1=t[:])
                            if selft is not None:
                                cur = selft[:, w * F_self:w * F_self + F_out]
                                if first:
                                    nc.vector.tensor_copy(out=acc[:], in_=cur)
                                    first = False
                                else:
                                    nc.vector.tensor_add(out=acc[:], in0=acc[:],
                                                         in1=cur)
                            if first:
                                nc.vector.memset(acc[:], 0.0)
                            cb(w, acc)

            # ---- feature conv 1 matmuls: u_f1 = (dinv*x) @ W0 ----
            for w0 in range(0, NWIN, 2):
                nwt = min(2, NWIN - w0)
                for a in range(nwt):
                    w = w0 + a
                    xt = wk.tile([P, IN_DIM], f32, tag="xt")
                    nc.sync.dma_start(out=xt[:], in_=x_sh[w * P:(w + 1) * P, :])
                    nc.vector.tensor_scalar(out=xt[:], in0=xt[:],
                                            scalar1=dinvb[:, w:w + 1],
                                            scalar2=None, op0=mybir.AluOpType.mult)
                    tp = ps.tile([P, P], f32, tag="tps")
                    nc.tensor.transpose(out=tp[:], in_=xt[:], identity=ident[:])
                    vT = wk.tile([P, P], f32, tag="vT")
                    nc.scalar.copy(out=vT[:], in_=tp[:])
                    up = ps.tile([P, HID], f32, tag="ups")
                    nc.tensor.matmul(out=up[:], lhsT=vT[:], rhs=W0[:], start=True,
                                     stop=True)
                    nc.scalar.copy(out=uselfF[:, w * HID:(w + 1) * HID], in_=up[:])
                nc.sync.dma_start(
                    out=bnR1[w0 * P:(w0 + nwt) * P, 0:HID]
                        .rearrange("(w p) f -> p w f", p=P),
                    in_=uselfF[:, w0 * HID:(w0 + nwt) * HID]
                        .rearrange("p (w f) -> p w f", w=nwt))
            nc.gpsimd.collective_compute(
                "AllGather", bass.mybir.AluOpType.bypass,
                replica_groups=[list(range(NC))],
                ins=[bnR1[:, :].opt()], outs=[tabR1[0:TABROWS, :].opt()])

            # ---- R1 aggregate: feature conv1 + statics C_all / Cm ----
            def cb_f1(w, acc):
                nc.vector.tensor_scalar(out=vFh[:, w * HID:(w + 1) * HID],
                                        in0=acc[:], scalar1=recipb[:, w:w + 1],
                                        scalar2=None, op0=mybir.AluOpType.mult)
                if bias0 is not None:
                    dv = wk.tile([P, HID], f32, tag="dbv")
                    nc.vector.tensor_scalar(out=dv[:], in0=bias0[:],
                                            scalar1=dinvb[:, w:w + 1],
                                            scalar2=None,
                                            op0=mybir.AluOpType.mult)
                    nc.vector.tensor_add(out=vFh[:, w * HID:(w + 1) * HID],
                                         in0=vFh[:, w * HID:(w + 1) * HID],
                                         in1=dv[:])

            def cb_call(w, acc):
                wl = w - wcut
                nc.vector.tensor_copy(out=G[:, wl * OUT:(wl + 1) * OUT],
                                      in_=acc[:])

            def cb_cm(w, acc):
                wl = w - wcut
                nc.vector.tensor_copy(out=Cstat[:, wl * OUT:(wl + 1) * OUT],
                                      in_=acc[:])

            stage_agg(tabR1, layF, 128,
                      [(0, 64, uselfF, HID, 0, cb_f1),
                       (64, 96, None, 0, wcut, cb_call),
                       (96, 128, None, 0, wcut, cb_cm)], idxF_d)

            # ---- feature conv 2 matmuls + AG (independent of label math) ----
            def post_f2(w, up):
                nc.scalar.copy(out=uselfF[:, w * HID:(w + 1) * HID], in_=up[:])
            stage_matmul(lambda w0, nwt: vFh[:, w0 * HID:(w0 + nwt) * HID],
                         HID, W1[:, :], HID, range(NWIN), post_f2,
                         uselfF, 0, bnF2, 0)
            nc.gpsimd.collective_compute(
                "AllGather", bass.mybir.AluOpType.bypass,
                replica_groups=[list(range(NC))],
                ins=[bnF2[:, :].opt()], outs=[tabF2[0:TABROWS, :].opt()])

            # ---- label rounds ----
            def label_round(j):
                """one label conv: S = gsrc(+extras) -> @Wl_j -> vtab/xlfin."""
                vt_new = vtab[(j + 1) % 2]
                last = (j == NUM_LBL - 1)

                def vsrc(w0, nwt):
                    st = wk.tile([P, 3 * OUT], f32, tag="S")
                    for a in range(nwt):
                        w = w0 + a
                        wl = w - wcut
                        dst = st[:, a * OUT:(a + 1) * OUT]
                        if j == 0:
                            nc.vector.tensor_add(
                                out=dst, in0=G[:, wl * OUT:(wl + 1) * OUT],
                                in1=dinvy[:, w * OUT:(w + 1) * OUT])
                        else:
                            nc.vector.tensor_add(
                                out=dst, in0=G[:, wl * OUT:(wl + 1) * OUT],
                                in1=Cstat[:, wl * OUT:(wl + 1) * OUT])
                            nc.vector.tensor_add(
                                out=dst, in0=dst,
                                in1=vtab[j % 2][:, wl * OUT:(wl + 1) * OUT])
                    return st[:, 0:nwt * OUT]

                def post(w, up):
                    wl = w - wcut
                    if last:
                        dstv = xlfin[:, w * OUT:(w + 1) * OUT]
                        nc.vector.tensor_scalar(out=dstv, in0=up[:],
                                                scalar1=dinvb[:, w:w + 1],
                                                scalar2=None,
                                                op0=mybir.AluOpType.mult)
                        nc.vector.copy_predicated(
                            out=dstv,
                            mask=maskb[:, w:w + 1].to_broadcast([P, OUT]),
                            data=yb[:, w * OUT:(w + 1) * OUT])
                    else:
                        nc.vector.tensor_scalar(
                            out=vt_new[:, wl * OUT:(wl + 1) * OUT],
                            in0=up[:], scalar1=umr[:, w:w + 1],
                            scalar2=None, op0=mybir.AluOpType.mult)

                stage_matmul(vsrc, OUT, Wl[:, j * OUT:(j + 1) * OUT], OUT,
                             range(wcut, NWIN), post,
                             None if last else vt_new, wcut,
                             None if last else bnL[(j + 1) % 2], 0)
                if not last:
                    nc.gpsimd.collective_compute(
                        "AllGather", bass.mybir.AluOpType.bypass,
                        replica_groups=[list(range(NC))],
                        ins=[bnL[(j + 1) % 2][base_l:NPAD, :].opt()],
                        outs=[tabL[(j + 1) % 2][0:tabrows_l, :].opt()])

            def cb_gv(w, acc):
                wl = w - wcut
                nc.vector.tensor_copy(out=G[:, wl * OUT:(wl + 1) * OUT],
                                      in_=acc[:])

            label_round(0)              # uses G = C_all; emits AG(vtab_1)

            # ---- feature conv 2 aggregate (overlaps label AG) ----
            def cb_f2(w, acc):
                dst = vFh[:, w * HID:(w + 1) * HID]
                nc.vector.tensor_scalar(out=dst, in0=acc[:],
                                        scalar1=dinvb[:, w:w + 1], scalar2=None,
                                        op0=mybir.AluOpType.mult)
                if bias1 is not None:
                    nc.vector.tensor_add(out=dst, in0=dst, in1=bias1[:])
            stage_agg(tabF2, layF, 64, [(0, 64, uselfF, HID, 0, cb_f2)], idxF_d)

            for j in range(1, NUM_LBL):
                stage_agg(tabL[j % 2], layL, 64,
                          [(0, 32, None, 0, wcut, cb_gv)], idxL_d)
                label_round(j)

            # ---- fuse: sigmoid([h | xl | dw] @ Wf + bf) ----
            for w in range(NWIN):
                dwt = wk.tile([P, DW], f32, tag="dwt")
                nc.sync.dma_start(out=dwt[:], in_=dw_sh[w * P:(w + 1) * P, :])
                fTa = wk.tile([P, P], f32, tag="fTa")
                fTb = wk.tile([DW - 32, P], f32, tag="fTb")
                tp = ps.tile([P, P], f32, tag="tps")
                nc.tensor.transpose(out=tp[0:HID, :],
                                    in_=vFh[:, w * HID:(w + 1) * HID],
                                    identity=ident[:])
                nc.scalar.copy(out=fTa[0:HID, :], in_=tp[0:HID, :])
                tp2 = ps.tile([P, P], f32, tag="tps")
                nc.tensor.transpose(out=tp2[0:OUT, :],
                                    in_=xlfin[:, w * OUT:(w + 1) * OUT],
                                    identity=ident[:])
                nc.scalar.copy(out=fTa[HID:HID + OUT, :], in_=tp2[0:OUT, :])
                tp3 = ps.tile([P, P], f32, tag="tps")
                nc.tensor.transpose(out=tp3[0:DW, :], in_=dwt[:],
                                    identity=ident[:])
                nc.scalar.copy(out=fTa[HID + OUT:P, :],
                               in_=tp3[0:P - HID - OUT, :])
                nc.scalar.copy(out=fTb[:, :], in_=tp3[P - HID - OUT:DW, :])
                op = ps.tile([P, OUT], f32, tag="ops")
                nc.tensor.matmul(out=op[:], lhsT=fTa[:], rhs=Wfa[:],
                                 start=True, stop=False)
                nc.tensor.matmul(out=op[:], lhsT=fTb[:], rhs=Wfb[:],
                                 start=False, stop=True)
                ot = wk.tile([P, OUT], f32, tag="ot")
                if biasf is not None:
                    nc.vector.tensor_add(out=op[:], in0=op[:], in1=biasf[:])
                nc.scalar.activation(out=ot[:], in_=op[:],
                                     func=bass.mybir.ActivationFunctionType.Sigmoid)
                nc.sync.dma_start(out=out_sh[w * P:(w + 1) * P, :], in_=ot[:])

    nc.compile()
    return nc


_CACHE = {}


def kernel(x, y, edge_index, deep_walk_emb, label_input_mask,
           W_gcn0, b_gcn0, W_gcn1, b_gcn1, W_label, b_label, W_fuse, b_fuse):
    import concourse.bass_utils as bass_utils

    n_nodes = x.shape[0]
    ei = np.asarray(edge_index, dtype=np.int64)
    mk = np.asarray(label_input_mask, bool)
    meta = _preprocess(ei, mk, n_nodes)
    core_of, local_of = meta["core_of"], meta["local_of"]

    nonzero_b = (bool(np.any(np.asarray(b_gcn0))),
                 bool(np.any(np.asarray(b_gcn1))),
                 bool(np.any(np.asarray(b_label))),
                 bool(np.any(np.asarray(b_fuse))))
    if nonzero_b[2]:
        raise NotImplementedError("nonzero label bias not wired")

    key = ("k3", n_nodes, ei.shape[1], nonzero_b, int(mk.sum()))
    if key not in _CACHE:
        _CACHE[key] = _build(meta, nonzero_b)
    nc = _CACHE[key]

    x_s = _shard_nodes(x, core_of, local_of, IN_DIM)
    y_s = _shard_nodes(y, core_of, local_of, OUT)
    dw_s = _shard_nodes(deep_walk_emb, core_of, local_of, DW)
    mk_s = _shard_nodes(mk.astype(np.int8)[:, None],
                       core_of, local_of, 1, dtype=np.int8)
    dg_s = np.zeros((NC, NPAD, 1), np.int32)
    dg_s[core_of, local_of, 0] = meta["deg"].astype(np.int32)

    bmax = max(HID, OUT) * NUM_LBL
    b_all = np.zeros((4, bmax), np.float32)
    b_all[0, :HID] = np.asarray(b_gcn0, np.float32)
    b_all[1, :HID] = np.asarray(b_gcn1, np.float32)
    b_all[2, :OUT * NUM_LBL] = np.asarray(b_label, np.float32).reshape(-1)
    b_all[3, :OUT] = np.asarray(b_fuse, np.float32)

    Wl_flat = np.asarray(W_label, np.float32).reshape(NUM_LBL * OUT, OUT)
    idxF128 = np.tile(meta["layF"]["idx"], (1, 8, 1))
    idxL128 = np.tile(meta["layL"]["idx"], (1, 8, 1))

    in_maps = []
    for c in range(NC):
        in_maps.append({
            "x_sh": x_s[c], "y_sh": y_s[c], "dw_sh": dw_s[c],
            "mask_sh": mk_s[c], "deg_sh": dg_s[c],
            "idxF_d": idxF128[c], "idxL_d": idxL128[c],
            "W0": np.asarray(W_gcn0, np.float32),
            "W1": np.asarray(W_gcn1, np.float32),
            "Wl": Wl_flat,
            "Wf": np.asarray(W_fuse, np.float32),
            "b_all": b_all,
        })
    res = bass_utils.run_bass_kernel_spmd(nc, in_maps, core_ids=list(range(NC)))
    out = np.empty((n_nodes, OUT), np.float32)
    for c in range(NC):
        sel = core_of == np.int64(c)
        out[sel] = res.results[c]["out_sh"][local_of[sel]]
    return out
